# revision 17
# baseline (speedup 1.0000x reference)
"""Trainium2 Bass kernel for nn_GBM_68539088109972 (v4: fp8 enc/dec, no dbc AR).

Encoder (32768->1024) -> Mamba block (d_inner=2048, selective scan L=256) ->
Decoder (1024->32768), B=2, L=256, distributed over 8 NeuronCores:
  - encoder : K-parallel over FLAT, fp8 DoubleRow (AllReduce lat per wave)
  - mamba   : u-branch of in_proj fully replicated (host permutes d_inner so
              each core's scan shard sits at tiles 0-1), so dbc / dt / B / C
              are all computed locally -- no dbc collective.  Scan + out_proj
              stay tensor-parallel over d_inner (AllReduce out_proj partials).
  - decoder : M-parallel over FLAT rows, fp8 DoubleRow; host gathers.

4 collectives total (2 per wave).  fp8 weights are scaled on host (enc_w*2^12,
dec_w*2^11, h*2^5 on-chip) and descaled in the PSUM-evacuating activations.
"""

import sys

sys.path.insert(0, "/opt/trn_rl_repo")

import numpy as np
import ml_dtypes

import concourse.bass as bass
import concourse.tile as tile
from concourse import bacc, mybir
from concourse import bass_utils

BF16 = ml_dtypes.bfloat16
FP8 = ml_dtypes.float8_e3m4
N_CORES = 8
B, L = 2, 256
BL = B * L                      # 512
H, W = 256, 128
FLAT = H * W                    # 32768
D_MODEL = 1024
D_INNER = 2048
D_STATE = 16
D_CONV = 4
DT_RANK = 64
EPS = 1e-5

P = 128
FLAT_SH = FLAT // N_CORES       # 4096
DI_SH = D_INNER // N_CORES      # 256
KE = FLAT_SH // P               # 32 encoder K tiles
MD = D_MODEL // P               # 8
TI = DI_SH // P                 # 2 own d_inner tiles per core
NT = D_INNER // P               # 16 full d_inner tiles
MF = FLAT_SH // P               # 32 decoder M tiles
KC = 4                          # enc K tiles per DMA chunk
NKB = KE // KC                  # 8 enc weight chunks
SL = L                          # tokens per wave (= one sequence)

S_ENC = 2.0 ** 8                # enc_w fp8 scale
S_DEC = 2.0 ** 6                # dec_w fp8 scale

f32 = mybir.dt.float32
bf16 = mybir.dt.bfloat16
fp8 = mybir.dt.float8e3

Act = mybir.ActivationFunctionType
Alu = mybir.AluOpType

# packed f32 const blob column offsets (enc_b is pre-divided by N_CORES and
# folded into the encoder PSUM evacuation, so lat post-AR includes enc_b)
C_ENCB = 0              # 8
C_DTB = 8               # 2
C_A = 10                # 32 (t*16+n)
C_D = 42                # 2
C_DECB = 44             # 32
C_CONVB = 76            # 16
C_TOT = 92
# packed bf16 blob column offsets
B_XP = 0                # 16*96 (x_proj, permuted full)
B_DTP = 1536            # 256
B_IPZ = 1792            # 8*256 (z in_proj shard)
B_OP = 3840             # 2*1024
B_CW = 5888             # 64 (conv taps, k*NT+t, permuted full)
B_CB = 5952             # 16 (conv bias)
B_TOT = 5968


def _body(tc, io, use_ar=True, phase='all'):
    nc = tc.nc
    RG = [list(range(N_CORES))]

    from contextlib import ExitStack
    ctx = ExitStack()
    const = ctx.enter_context(tc.tile_pool(name="const", bufs=1))
    xpool = ctx.enter_context(tc.tile_pool(name="xpool", bufs=1))
    wbig = ctx.enter_context(tc.tile_pool(name="wbig", bufs=8))
    wup = ctx.enter_context(tc.tile_pool(name="wup", bufs=1))
    latp = ctx.enter_context(tc.tile_pool(name="latp", bufs=1))
    scanp = ctx.enter_context(tc.tile_pool(name="scanp", bufs=1))
    ubigp = ctx.enter_context(tc.tile_pool(name="ubig", bufs=1))
    big = ctx.enter_context(tc.tile_pool(name="big", bufs=2))
    outp = ctx.enter_context(tc.tile_pool(name="outp", bufs=2))
    psum = ctx.enter_context(tc.tile_pool(name="psum", bufs=8, space="PSUM"))
    dram = ctx.enter_context(tc.tile_pool(name="dram", bufs=1, space="DRAM"))

    def ar(kind_in, kind_out):
        if use_ar:
            nc.gpsimd.collective_compute(
                "AllReduce", Alu.add, replica_groups=RG,
                ins=[kind_in.opt()], outs=[kind_out.opt()])
        else:
            nc.sync.dma_start(kind_out[:], kind_in[:])

    # ---------------- consts ----------------
    cp = const.tile([P, C_TOT], f32, name="cpack")
    nc.sync.dma_start(cp[:], io["cpack"][:, :])

    def c1(off):
        return cp[:, off:off + 1]

    ones_sb = const.tile([P, 1], bf16, name="ones")
    nc.vector.memset(ones_sb[:], 1.0)
    ones_row = const.tile([1, P], f32, name="onesrow")
    nc.vector.memset(ones_row[:], 1.0)
    onesf_sb = const.tile([P, 1], f32, name="onesf")
    nc.vector.memset(onesf_sb[:], 1.0)
    eps_sb = const.tile([1, 1], f32, name="eps")
    nc.vector.memset(eps_sb[:], EPS)

    # ---------------- x + enc_w streamed in (enc_w slots reused by dec_w) --
    x_sb = []
    w_enc = []
    for kb in range(NKB):
        xk = xpool.tile([P, KC, BL], fp8, name=f"xk{kb}")
        nc.sync.dma_start(xk[:], io["xT"][:, kb * KC:(kb + 1) * KC, :])
        x_sb.append(xk)
        wk = wbig.tile([P, KC, D_MODEL], fp8, tag="w", bufs=8)
        nc.sync.dma_start(wk[:], io["enc_wT"][:, kb * KC:(kb + 1) * KC, :])
        w_enc.append(wk)

    bp = const.tile([P, B_TOT], bf16, name="bpack")
    nc.sync.dma_start(bp[:], io["bpack"][:, :])
    upk = wup.tile([P, MD, D_INNER], bf16, name="upack")
    nc.sync.dma_start(upk[:], io["upack"][:, :, :])

    # per-wave state dicts
    wv = [dict() for _ in range(B)]
    for w in range(B):
        d = wv[w]
        d["sl"] = slice(w * SL, (w + 1) * SL)
        d["ar1i"] = dram.tile([P, MD, SL], bf16, name=f"ar1i{w}")
        d["ar1o"] = dram.tile([P, MD, SL], bf16, name=f"ar1o{w}",
                              addr_space="Shared")
        d["ar3i"] = dram.tile([P, MD, SL], bf16, name=f"ar3i{w}")
        d["ar3o"] = dram.tile([P, MD, SL], bf16, name=f"ar3o{w}",
                              addr_space="Shared")
        d["bc_dr"] = dram.tile([2 * D_STATE, SL], bf16, name=f"bcdr{w}")

    # ============ encoder (fp8e3 weights, kb-outer streaming) ============
    def enc(w):
        d = wv[w]
        sl = d["sl"]
        d["latpar"] = latp.tile([P, MD, SL], bf16, name=f"latpar{w}")
        lp = d["latpar"]
        # 4 m-tiles in flight, each in its OWN psum bank (start= clears accum
        # bits bank-wide, so concurrently-accumulating tiles must not share)
        for g in range(2):
            ps = [psum.tile([P, 2, SL], f32, name=f"encps{w}_{g}_{i}",
                            tag="encps", bufs=4) for i in range(4)]
            for kb in range(NKB):
                for k4 in range(KC):
                    for mi in range(4):
                        m = g * 4 + mi
                        nc.tensor.matmul(
                            ps[mi][:, 0, :],
                            lhsT=w_enc[kb][:, k4, m * P:(m + 1) * P],
                            rhs=x_sb[kb][:, k4, sl],
                            start=(kb == 0 and k4 == 0),
                            stop=(kb == NKB - 1 and k4 == KC - 1),
                        )
            for mi in range(4):
                m = g * 4 + mi
                nc.scalar.activation(lp[:, m, :], ps[mi][:, 0, :],
                                     Act.Copy, scale=1.0 / S_ENC)
        nc.sync.dma_start(d["ar1i"][:], lp[:])

    # ================= rmsnorm (reload lat, rstd broadcast) =================
    def rms(w):
        d = wv[w]
        lch = latp.tile([P, MD, SL], bf16, name=f"lbfc{w}")
        nc.scalar.dma_start(lch[:], d["ar1o"][:])
        if not ZB_ENC:
            nc.vector.tensor_tensor(
                lch[:], lch[:], cp[:, C_ENCB:C_ENCB + MD][:, :, None]
                .to_broadcast((P, MD, SL)), Alu.add)
        d["lch"] = lch
        lat_bf = [lch[:, m, :] for m in range(MD)]
        d["lat_bf"] = lat_bf
        ss_t = psum.tile([P, SL], f32, name=f"ssps{w}", tag="mm", bufs=2)
        ss_ps = ss_t[0:1, :]
        for m in range(MD):
            sq = scanp.tile([P, SL], bf16, name="sq", tag="sq", bufs=2)
            nc.scalar.activation(sq[:], lat_bf[m], Act.Square)
            nc.tensor.matmul(ss_ps, lhsT=ones_sb[:], rhs=sq[:],
                             start=(m == 0), stop=(m == MD - 1))
        # rstd = exp(-0.5 * ln(ss/D + eps)) (stays in the ln/exp act set)
        lms = scanp.tile([1, SL], f32, name="lms", tag="rmssc", bufs=4)
        nc.scalar.activation(lms[:], ss_ps, Act.Ln, bias=eps_sb[:],
                             scale=1.0 / D_MODEL)
        rstd = scanp.tile([1, SL], f32, name="rstd", tag="rmssc", bufs=4)
        nc.scalar.activation(rstd[:], lms[:], Act.Exp, scale=-0.5)
        rstd_ps = psum.tile([P, SL], f32, name=f"rstdps{w}", tag="mm", bufs=2)
        nc.tensor.matmul(rstd_ps[:], lhsT=ones_row[:], rhs=rstd[:],
                         start=True, stop=True)
        rstd_bc = scanp.tile([P, SL], f32, name=f"rstdbc{w}", tag="rstdbc",
                             bufs=2)
        nc.vector.tensor_copy(rstd_bc[:], rstd_ps[:])
        d["rstd_bc"] = rstd_bc

    # ====== mamba front: full-u in_proj (replicated), z (shard) ======
    def inproj(w):
        d = wv[w]
        uf = ubigp.tile([P, NT, SL], bf16, name=f"ufull{w}", tag="ufull",
                        bufs=1)
        d["uraw"] = uf
        for grp in range(NT // 2):
            ups = psum.tile([P, 2, SL], f32, name=f"ups{w}_{grp}", tag="uzps",
                            bufs=2)
            for i in range(2):
                t = grp * 2 + i
                for k in range(MD):
                    nc.tensor.matmul(
                        ups[:, i, :],
                        lhsT=upk[:, k, t * P:(t + 1) * P],
                        rhs=d["lat_bf"][k], start=(k == 0), stop=(k == MD - 1))
            for i in range(2):
                nc.vector.tensor_tensor(uf[:, grp * 2 + i, :], ups[:, i, :],
                                        d["rstd_bc"][:], Alu.mult)
        zps = psum.tile([P, 2, SL], f32, name=f"zps{w}", tag="uzps", bufs=2)
        for i in range(TI):
            for k in range(MD):
                nc.tensor.matmul(
                    zps[:, i, :],
                    lhsT=bp[:, B_IPZ + k * DI_SH + i * P:
                            B_IPZ + k * DI_SH + (i + 1) * P],
                    rhs=d["lat_bf"][k], start=(k == 0), stop=(k == MD - 1))
        zn = scanp.tile([P, TI, SL], f32, name="zn", tag="zn", bufs=1)
        for i in range(TI):
            nc.vector.tensor_tensor(zn[:, i, :], zps[:, i, :],
                                    d["rstd_bc"][:], Alu.mult)
        sz = scanp.tile([P, TI, SL], f32, name=f"siluz{w}", tag="sz", bufs=1)
        nc.scalar.activation(sz[:], zn[:], Act.Silu)
        d["silu_z"] = sz

    # ====== causal depthwise conv + silu over the full (permuted) u ======
    def conv_silu(w):
        d = wv[w]
        uf = d["uraw"]
        ub = ubigp.tile([P, NT, SL], bf16, name=f"ubf{w}", tag="ubf", bufs=1)
        d["u_bf"] = ub

        def cw_bc(k, hh, n):
            return bp[:, B_CW + k * NT + hh * 8:B_CW + k * NT + hh * 8 + 8]                 [:, :, None].to_broadcast((P, 8, n))

        for hh in range(NT // 8):
            s8 = slice(hh * 8, hh * 8 + 8)
            acc = ubigp.tile([P, 8, SL], bf16, name="cacc", tag="cacc", bufs=1)
            nc.vector.tensor_tensor(acc[:], uf[:, s8, :], cw_bc(3, hh, SL),
                                    Alu.mult)
            nc.vector.tensor_tensor(
                acc[:], acc[:], bp[:, B_CB + hh * 8:B_CB + hh * 8 + 8]
                [:, :, None].to_broadcast((P, 8, SL)), Alu.add)
            for k in range(3):
                s = 3 - k
                tmp = ubigp.tile([P, 8, SL], bf16, name="ctmp", tag="ctmp",
                                 bufs=1)
                nc.vector.tensor_tensor(tmp[:, :, s:], uf[:, s8, 0:SL - s],
                                        cw_bc(k, hh, SL - s), Alu.mult)
                nc.vector.tensor_tensor(acc[:, :, s:], acc[:, :, s:],
                                        tmp[:, :, s:], Alu.add)
            nc.scalar.activation(ub[:, s8, :], acc[:], Act.Silu)

    # ====== x_proj (local, contraction over full u) + dt + B/C ======
    def xproj(w):
        d = wv[w]
        dbc_t = psum.tile([P, SL], f32, name=f"dbcps{w}", tag="mm", bufs=2)
        dbc_ps = dbc_t[0:96, :]
        for t in range(NT):
            nc.tensor.matmul(dbc_ps, lhsT=bp[:, B_XP + t * 96:
                                             B_XP + (t + 1) * 96],
                             rhs=d["u_bf"][:, t, :], start=(t == 0),
                             stop=(t == NT - 1))
        dbc_bf = scanp.tile([P, SL], bf16, name="dbcbf", tag="dbcbf", bufs=2)
        nc.vector.memset(dbc_bf[:], 0.0)
        nc.vector.tensor_copy(dbc_bf[0:DT_RANK, :], dbc_ps[0:DT_RANK, :])
        bc_bf = scanp.tile([2 * D_STATE, SL], bf16, name="bcbf", tag="bcbf",
                           bufs=2)
        nc.vector.tensor_copy(bc_bf[:], dbc_ps[DT_RANK:DT_RANK + 2 * D_STATE, :])
        nc.sync.dma_start(d["bc_dr"][:], bc_bf[:])

        dt_t = []
        for t in range(TI):
            ps = psum.tile([P, SL], f32, name=f"dtps{w}_{t}", tag="mm", bufs=2)
            nc.tensor.matmul(ps[:], lhsT=bp[:, B_DTP + t * P:
                                            B_DTP + (t + 1) * P],
                             rhs=dbc_bf[:], start=True, stop=True)
            # softplus(x+b) = log1p(exp(x+b)); args well within range
            edt = scanp.tile([P, SL], f32, name="edt", tag="edt", bufs=1)
            nc.scalar.activation(edt[:], ps[:], Act.Exp, bias=c1(C_DTB + t))
            dtt = scanp.tile([P, SL], f32, name=f"dt{w}_{t}", tag="dtt", bufs=4)
            nc.scalar.activation(dtt[:], edt[:], Act.Ln, bias=onesf_sb[:])
            dt_t.append(dtt)
        d["dt_t"] = dt_t

    # ================= selective scan + gate =================
    def scan(w):
        d = wv[w]
        dt_t = d["dt_t"]
        NH = D_STATE // 4           # 4 states per quarter
        NLH = NH * SL               # 1024
        # one broadcast DMA for all of B, one for all of C
        bctB = big.tile([P, D_STATE, SL], bf16, name=f"bctB{w}", tag="bctB",
                        bufs=1)
        nc.scalar.dma_start(bctB[:], d["bc_dr"][0:D_STATE, :][None, :, :]
                            .to_broadcast((P, D_STATE, SL)))
        bctC = big.tile([P, D_STATE, SL], bf16, name=f"bctC{w}", tag="bctC",
                        bufs=1)
        nc.scalar.dma_start(bctC[:], d["bc_dr"][D_STATE:2 * D_STATE, :]
                            [None, :, :].to_broadcast((P, D_STATE, SL)))

        y_t = [scanp.tile([P, SL], f32, name=f"y{w}_{t}", tag="yt", bufs=2)
               for t in range(TI)]
        for t in range(TI):
            du = scanp.tile([P, SL], f32, name="du", tag="du", bufs=2)
            nc.vector.tensor_tensor(du[:], dt_t[t][:], d["u_bf"][:, t, :],
                                    Alu.mult)
            yh = scanp.tile([P, SL], f32, name="yh", tag="du", bufs=2)
            for hf in range(2):
                n0 = hf * 8
                dA = big.tile([P, 8 * SL], bf16, name="dA", tag="dAh", bufs=2)
                dAv = dA[:].rearrange("p (n l) -> p n l", n=8)
                for n in range(8):
                    nc.scalar.activation(dAv[:, n, :], dt_t[t][:], Act.Exp,
                                         scale=cp[:, C_A + t * 16 + n0 + n:
                                                  C_A + t * 16 + n0 + n + 1])
                nc.vector.memset(dAv[:, :, 0:1], 0.0)

                dBu = big.tile([P, 8 * SL], bf16, name="dBu", tag="dBuh",
                               bufs=2)
                nc.gpsimd.tensor_tensor(
                    dBu[:].rearrange("p (n l) -> p n l", n=8),
                    du[:, None, :].to_broadcast((P, 8, SL)),
                    bctB[:, n0:n0 + 8, :], Alu.mult)

                h = big.tile([P, 8 * SL], bf16, name="h", tag="hh", bufs=1)
                nc.vector.tensor_tensor_scan(h[:], dA[:], dBu[:], 0.0,
                                             Alu.mult, Alu.add)

                hC = big.tile([P, 8 * SL], bf16, name="hC", tag="dBuh",
                              bufs=2)
                nc.gpsimd.tensor_tensor(hC[:], h[:], bctC[:, n0:n0 + 8, :]
                                        .rearrange("p n l -> p (n l)"),
                                        Alu.mult)
                tgt = y_t[t][:] if hf == 0 else yh[:]
                nc.vector.tensor_reduce(
                    tgt, hC[:].rearrange("p (n l) -> p l n", n=8),
                    axis=mybir.AxisListType.X, op=Alu.add)
                if hf > 0:
                    nc.vector.tensor_tensor(y_t[t][:], y_t[t][:], yh[:],
                                            Alu.add)
            nc.vector.scalar_tensor_tensor(
                out=y_t[t][:], in0=d["u_bf"][:, t, :], scalar=c1(C_D + t),
                in1=y_t[t][:], op0=Alu.mult, op1=Alu.add)
        d["y_t"] = y_t

        y_bf = []
        for t in range(TI):
            yb16 = scanp.tile([P, SL], bf16, name=f"ybf{w}_{t}", tag="ybf",
                              bufs=2)
            nc.vector.tensor_tensor(yb16[:], y_t[t][:],
                                    d["silu_z"][:, t, :], Alu.mult)
            y_bf.append(yb16)
        d["y_bf"] = y_bf

    def outproj(w):
        d = wv[w]
        hp = latp.tile([P, MD, SL], bf16, name=f"hppar{w}")
        d["hppar"] = hp
        for m in range(MD):
            ps = psum.tile([P, SL], f32, name="mmps", tag="mm", bufs=2)
            for t in range(TI):
                nc.tensor.matmul(
                    ps[:], lhsT=bp[:, B_OP + t * D_MODEL + m * P:
                                   B_OP + t * D_MODEL + (m + 1) * P],
                    rhs=d["y_bf"][t][:], start=(t == 0), stop=(t == TI - 1))
            nc.scalar.activation(hp[:, m, :], ps[:], Act.Copy)
        nc.sync.dma_start(d["ar3i"][:], hp[:])

    # ================= decoder (fp8 DoubleRow) =================
    w_dec = []

    def dec_prefetch():
        for mp in range(NKB):
            dwm = wbig.tile([P, KC, MD, P], fp8, tag="w", bufs=8)
            nc.sync.dma_start(dwm[:],
                              io["dec_wT"][:, KC * mp:KC * mp + KC, :, :])
            w_dec.append(dwm)

    def dec_h(w):
        d = wv[w]
        # reuse dead staging tiles: hppar (after ar3i DMA) for the reload,
        # latpar (after ar1i DMA) for the residual-summed h
        hch = d["hppar"]
        nc.scalar.dma_start(hch[:], d["ar3o"][:])
        lp = d["latpar"]
        nc.vector.tensor_tensor(lp[:], hch[:], d["lch"][:], Alu.add)
        d["h_bf"] = lp

    def dec(w, mps):
        d = wv[w]
        hb = d["h_bf"]
        for mp2 in mps:                     # mp2 indexes pairs of m-chunks
            ot = outp.tile([P, 2 * KC, SL], bf16, name="ot", tag="ot", bufs=2)
            for half in range(2):
                mp = 2 * mp2 + half
                for mi in range(KC):
                    m = KC * mp + mi
                    ps = psum.tile([P, SL], f32, name="mmps", tag="mm", bufs=2)
                    for k in range(MD):
                        nc.tensor.matmul(
                            ps[:], lhsT=w_dec[mp][:, mi, k, :],
                            rhs=hb[:, k, :], start=(k == 0),
                            stop=(k == MD - 1))
                    nc.scalar.activation(ot[:, half * KC + mi, :], ps[:],
                                         Act.Sigmoid, bias=c1(C_DECB + m),
                                         scale=1.0 / S_DEC)
            nc.sync.dma_start(
                io["out"][w, :, 2 * KC * mp2:2 * KC * (mp2 + 1), :], ot[:])

    # ================= emission order (the pipeline) =================
    enc(0)
    ar(wv[0]["ar1i"], wv[0]["ar1o"])
    enc(1)
    ar(wv[1]["ar1i"], wv[1]["ar1o"])
    dec_prefetch()
    rms(0)
    inproj(0)
    conv_silu(0)
    xproj(0)
    if phase == 'enc':
        rms(1)
        for w in range(B):
            d = wv[w]
            for m in range(MD):
                nc.sync.dma_start(io["out"][w, :, m, :], d["lat_bf"][m])
        ctx.close()
        return
    scan(0)
    rms(1)
    inproj(1)
    outproj(0)
    ar(wv[0]["ar3i"], wv[0]["ar3o"])
    conv_silu(1)
    xproj(1)
    scan(1)
    if phase == 'scan':
        for w in range(B):
            d = wv[w]
            for t in range(TI):
                nc.sync.dma_start(io["out"][w, :, t, :], d["y_bf"][t][:])
        ctx.close()
        return
    dec_h(0)
    dec(0, range(2))
    outproj(1)
    ar(wv[1]["ar3i"], wv[1]["ar3o"])
    dec(0, range(2, 4))
    dec_h(1)
    dec(1, range(4))
    ctx.close()


_CACHE = {}
ZB_ENC = True
ZB_CONV = True


def _get_compiled(repeat=1, use_ar=True, phase="all"):
    if ("nc", repeat, use_ar, phase, ZB_ENC, ZB_CONV) in _CACHE:
        return _CACHE[("nc", repeat, use_ar, phase, ZB_ENC, ZB_CONV)]
    nc = bacc.Bacc("TRN2", target_bir_lowering=False, debug=False,
                   num_devices=N_CORES)

    def inp(name, shape, dt=bf16):
        return nc.dram_tensor(name, list(shape), dt, kind="ExternalInput").ap()

    io = {
        "xT": inp("xT", (P, KE, BL), fp8),
        "enc_wT": inp("enc_wT", (P, KE, D_MODEL), fp8),
        "cpack": inp("cpack", (P, C_TOT), f32),
        "bpack": inp("bpack", (P, B_TOT)),
        "upack": inp("upack", (P, MD, D_INNER)),
        "dec_wT": inp("dec_wT", (P, MF, MD, P), fp8),
        "out": nc.dram_tensor("out", [B, P, MF, SL], bf16,
                              kind="ExternalOutput").ap(),
    }
    with tile.TileContext(nc) as tc:
        for _ in range(repeat):
            _body(tc, io, use_ar=use_ar, phase=phase)
    nc.compile()
    _CACHE[("nc", repeat, use_ar, phase, ZB_ENC, ZB_CONV)] = nc
    return nc


def _shard_inputs(x, enc_w, enc_b, dec_w, dec_b, norm_w, in_proj_w, conv_w,
                  conv_b, x_proj_w, dt_proj_w, dt_proj_b, A_log, D_skip,
                  out_proj_w):
    """Host-side preprocessing: transposes, folds, dtype casts, sharding."""
    def q8(a, s):
        a = a * np.float32(s)
        assert np.abs(a).max() < 15.4, np.abs(a).max()
        return a.astype(FP8)

    x2d = np.ascontiguousarray(x.reshape(BL, FLAT).T)          # (FLAT, BL)
    xT = x2d.astype(FP8)
    enc_wT = q8(np.ascontiguousarray(enc_w.T), S_ENC)          # (FLAT, D_MODEL)
    Wp = (in_proj_w * norm_w[None, :])                         # fold rmsnorm scale
    A = -np.exp(A_log).astype(np.float32)                      # (D_INNER, D_STATE)
    dt_projT = np.ascontiguousarray(dt_proj_w.T)               # (64, D_INNER)
    x_projT = np.ascontiguousarray(x_proj_w.T)                 # (D_INNER, 96)
    out_projT = np.ascontiguousarray(out_proj_w.T)             # (D_INNER, D_MODEL)
    conv_w2 = conv_w.reshape(D_CONV, D_INNER)                  # (4, D_INNER)
    Wu = Wp[:D_INNER]                                          # (D_INNER, D_MODEL)

    in_maps = []
    for i in range(N_CORES):
        fsl = slice(i * FLAT_SH, (i + 1) * FLAT_SH)
        dsl = slice(i * DI_SH, (i + 1) * DI_SH)
        # permutation putting this core's scan shard first
        perm = np.concatenate([np.arange(i * DI_SH, (i + 1) * DI_SH),
                               np.arange(0, i * DI_SH),
                               np.arange((i + 1) * DI_SH, D_INNER)])
        Wz = Wp[D_INNER + i * DI_SH: D_INNER + (i + 1) * DI_SH]
        z_projT = np.ascontiguousarray(Wz.T).astype(BF16)      # (D_MODEL, 256)
        dtp = np.zeros((P, DI_SH), np.float32)
        dtp[:DT_RANK] = dt_projT[:, dsl]

        cpack = np.zeros((P, C_TOT), np.float32)
        cpack[:, C_ENCB:C_ENCB + MD] = enc_b.reshape(MD, P).T
        cpack[:, C_DTB:C_DTB + TI] = dt_proj_b[dsl].reshape(TI, P).T
        cpack[:, C_CONVB:C_CONVB + NT] = conv_b[perm].reshape(NT, P).T
        cpack[:, C_A:C_A + 32] = \
            A[dsl].reshape(TI, P, D_STATE).transpose(1, 0, 2).reshape(P, 32)
        cpack[:, C_D:C_D + TI] = D_skip[dsl].reshape(TI, P).T
        cpack[:, C_DECB:C_DECB + MF] = dec_b[fsl].reshape(MF, P).T

        bpack = np.zeros((P, B_TOT), BF16)
        bpack[:, B_XP:B_XP + NT * 96] = \
            x_projT[perm].reshape(NT, P, 96).transpose(1, 0, 2) \
            .reshape(P, NT * 96).astype(BF16)
        bpack[:, B_DTP:B_DTP + DI_SH] = dtp.astype(BF16)
        bpack[:, B_IPZ:B_IPZ + MD * DI_SH] = \
            z_projT.reshape(MD, P, DI_SH).transpose(1, 0, 2) \
            .reshape(P, MD * DI_SH)
        bpack[:, B_OP:B_OP + TI * D_MODEL] = \
            out_projT[dsl].reshape(TI, P, D_MODEL).transpose(1, 0, 2) \
            .reshape(P, TI * D_MODEL).astype(BF16)
        cwp = conv_w2[:, perm].reshape(D_CONV, NT, P)          # (4, NT, P)
        bpack[:, B_CW:B_CW + 4 * NT] = \
            cwp.transpose(2, 0, 1).reshape(P, 4 * NT).astype(BF16)
        bpack[:, B_CB:B_CB + NT] = conv_b[perm].reshape(NT, P).T.astype(BF16)

        upack = np.ascontiguousarray(
            Wu[perm].T.reshape(MD, P, D_INNER).transpose(1, 0, 2)) \
            .astype(BF16)                                      # (P, MD, D_INNER)

        dec_wTi = q8(np.ascontiguousarray(
            dec_w[fsl].reshape(MF, P, MD, P).transpose(3, 0, 2, 1)), S_DEC)
        in_maps.append({
            "xT": np.ascontiguousarray(
                xT[fsl].reshape(KE, P, BL).transpose(1, 0, 2)),
            "enc_wT": np.ascontiguousarray(
                enc_wT[fsl].reshape(KE, P, D_MODEL).transpose(1, 0, 2)),
            "cpack": cpack,
            "bpack": bpack,
            "upack": upack,
            "dec_wT": dec_wTi,
        })
    return in_maps


def run(inputs, trace=False, tmpdir=None):
    """Run on hardware; returns (probs, BassKernelResults)."""
    global ZB_ENC, ZB_CONV
    ZB_ENC = bool(np.all(np.asarray(inputs["enc_b"]) == 0))
    ZB_CONV = bool(np.all(np.asarray(inputs["conv_b"]) == 0))
    in_maps = _shard_inputs(**{k: np.asarray(v) for k, v in inputs.items()})
    nc = _get_compiled()
    res = bass_utils.run_bass_kernel_spmd(
        nc, in_maps, core_ids=list(range(N_CORES)), trace=trace, tmpdir=tmpdir)
    parts = [res.results[c]["out"].transpose(0, 3, 2, 1)   # (B, SL, MF, P)
             .reshape(B, SL, FLAT_SH) for c in range(N_CORES)]
    full = np.concatenate(parts, axis=2).astype(np.float32)    # (B, L, FLAT)
    probs = full.reshape(B, L, H, W)[:, :L - 1]
    return probs, res


def kernel(**inputs):
    return run(inputs, trace=False)[0]


# revision 18
# speedup vs baseline: 1.0794x; 1.0794x over previous
"""Trainium2 Bass kernel for nn_GBM_68539088109972 (v4: fp8 enc/dec, no dbc AR).

Encoder (32768->1024) -> Mamba block (d_inner=2048, selective scan L=256) ->
Decoder (1024->32768), B=2, L=256, distributed over 8 NeuronCores:
  - encoder : K-parallel over FLAT, fp8 DoubleRow (AllReduce lat per wave)
  - mamba   : u-branch of in_proj fully replicated (host permutes d_inner so
              each core's scan shard sits at tiles 0-1), so dbc / dt / B / C
              are all computed locally -- no dbc collective.  Scan + out_proj
              stay tensor-parallel over d_inner (AllReduce out_proj partials).
  - decoder : M-parallel over FLAT rows, fp8 DoubleRow; host gathers.

4 collectives total (2 per wave).  fp8 weights are scaled on host (enc_w*2^12,
dec_w*2^11, h*2^5 on-chip) and descaled in the PSUM-evacuating activations.
"""

import sys

sys.path.insert(0, "/opt/trn_rl_repo")

import numpy as np
import ml_dtypes

import concourse.bass as bass
import concourse.tile as tile
from concourse import bacc, mybir
from concourse import bass_utils

BF16 = ml_dtypes.bfloat16
FP8 = ml_dtypes.float8_e3m4
N_CORES = 8
B, L = 2, 256
BL = B * L                      # 512
H, W = 256, 128
FLAT = H * W                    # 32768
D_MODEL = 1024
D_INNER = 2048
D_STATE = 16
D_CONV = 4
DT_RANK = 64
EPS = 1e-5

P = 128
FLAT_SH = FLAT // N_CORES       # 4096
DI_SH = D_INNER // N_CORES      # 256
KE = FLAT_SH // P               # 32 encoder K tiles
MD = D_MODEL // P               # 8
TI = DI_SH // P                 # 2 own d_inner tiles per core
NT = D_INNER // P               # 16 full d_inner tiles
MF = FLAT_SH // P               # 32 decoder M tiles
KC = 4                          # enc K tiles per DMA chunk
NKB = KE // KC                  # 8 enc weight chunks
SL = L                          # tokens per wave (= one sequence)

S_ENC = 2.0 ** 8                # enc_w fp8 scale
S_DEC = 2.0 ** 6                # dec_w fp8 scale

f32 = mybir.dt.float32
bf16 = mybir.dt.bfloat16
fp8 = mybir.dt.float8e3

Act = mybir.ActivationFunctionType
Alu = mybir.AluOpType

# packed f32 const blob column offsets (enc_b is pre-divided by N_CORES and
# folded into the encoder PSUM evacuation, so lat post-AR includes enc_b)
C_ENCB = 0              # 8
C_DTB = 8               # 2
C_A = 10                # 32 (t*16+n)
C_D = 42                # 2
C_DECB = 44             # 32
C_CONVB = 76            # 16
C_TOT = 92
# packed bf16 blob column offsets
B_XP = 0                # 16*96 (x_proj, permuted full)
B_DTP = 1536            # 256
B_IPZ = 1792            # 8*256 (z in_proj shard)
B_OP = 3840             # 2*1024
B_CW = 5888             # 64 (conv taps, k*NT+t, permuted full)
B_CB = 5952             # 16 (conv bias)
B_TOT = 5968


def _body(tc, io, use_ar=True, phase='all'):
    nc = tc.nc
    RG = [list(range(N_CORES))]

    from contextlib import ExitStack
    ctx = ExitStack()
    const = ctx.enter_context(tc.tile_pool(name="const", bufs=1))
    xpool = ctx.enter_context(tc.tile_pool(name="xpool", bufs=1))
    wbig = ctx.enter_context(tc.tile_pool(name="wbig", bufs=8))
    wup = ctx.enter_context(tc.tile_pool(name="wup", bufs=1))
    latp = ctx.enter_context(tc.tile_pool(name="latp", bufs=1))
    scanp = ctx.enter_context(tc.tile_pool(name="scanp", bufs=1))
    ubigp = ctx.enter_context(tc.tile_pool(name="ubig", bufs=1))
    big = ctx.enter_context(tc.tile_pool(name="big", bufs=2))
    outp = ctx.enter_context(tc.tile_pool(name="outp", bufs=2))
    psum = ctx.enter_context(tc.tile_pool(name="psum", bufs=8, space="PSUM"))
    dram = ctx.enter_context(tc.tile_pool(name="dram", bufs=1, space="DRAM"))

    def ar(kind_in, kind_out):
        if use_ar:
            nc.gpsimd.collective_compute(
                "AllReduce", Alu.add, replica_groups=RG,
                ins=[kind_in.opt()], outs=[kind_out.opt()])
        else:
            nc.sync.dma_start(kind_out[:], kind_in[:])

    # ---------------- consts ----------------
    cp = const.tile([P, C_TOT], f32, name="cpack")
    nc.sync.dma_start(cp[:], io["cpack"][:, :])

    def c1(off):
        return cp[:, off:off + 1]

    ones_sb = const.tile([P, 1], bf16, name="ones")
    nc.vector.memset(ones_sb[:], 1.0)
    ones_row = const.tile([1, P], f32, name="onesrow")
    nc.vector.memset(ones_row[:], 1.0)
    onesf_sb = const.tile([P, 1], f32, name="onesf")
    nc.vector.memset(onesf_sb[:], 1.0)
    eps_sb = const.tile([1, 1], f32, name="eps")
    nc.vector.memset(eps_sb[:], EPS)

    # ---------------- x + enc_w streamed in (enc_w slots reused by dec_w) --
    x_sb = []
    w_enc = []
    for kb in range(NKB):
        xk = xpool.tile([P, KC, BL], fp8, name=f"xk{kb}")
        nc.sync.dma_start(xk[:], io["xT"][:, kb * KC:(kb + 1) * KC, :])
        x_sb.append(xk)
        wk = wbig.tile([P, KC, D_MODEL], fp8, tag="w", bufs=8)
        nc.sync.dma_start(wk[:], io["enc_wT"][:, kb * KC:(kb + 1) * KC, :])
        w_enc.append(wk)

    bp = const.tile([P, B_TOT], bf16, name="bpack")
    nc.sync.dma_start(bp[:], io["bpack"][:, :])
    upk = wup.tile([P, MD, D_INNER], bf16, name="upack")
    nc.sync.dma_start(upk[:], io["upack"][:, :, :])

    # per-wave state dicts
    wv = [dict() for _ in range(B)]
    for w in range(B):
        d = wv[w]
        d["sl"] = slice(w * SL, (w + 1) * SL)
        d["ar1i"] = dram.tile([P, MD, SL], bf16, name=f"ar1i{w}")
        d["ar1o"] = dram.tile([P, MD, SL], bf16, name=f"ar1o{w}",
                              addr_space="Shared")
        d["ar3i"] = dram.tile([P, MD, SL], bf16, name=f"ar3i{w}")
        d["ar3o"] = dram.tile([P, MD, SL], bf16, name=f"ar3o{w}",
                              addr_space="Shared")
        d["bc_dr"] = dram.tile([2 * D_STATE, SL], bf16, name=f"bcdr{w}")

    # ============ encoder (fp8e3 weights, kb-outer streaming) ============
    def enc(w):
        d = wv[w]
        sl = d["sl"]
        d["latpar"] = latp.tile([P, MD, SL], bf16, name=f"latpar{w}")
        lp = d["latpar"]
        # 4 m-tiles in flight, each in its OWN psum bank (start= clears accum
        # bits bank-wide, so concurrently-accumulating tiles must not share)
        for g in range(2):
            ps = [psum.tile([P, 2, SL], f32, name=f"encps{w}_{g}_{i}",
                            tag="encps", bufs=4) for i in range(4)]
            for kb in range(NKB):
                for k4 in range(KC):
                    for mi in range(4):
                        m = g * 4 + mi
                        nc.tensor.matmul(
                            ps[mi][:, 0, :],
                            lhsT=w_enc[kb][:, k4, m * P:(m + 1) * P],
                            rhs=x_sb[kb][:, k4, sl],
                            start=(kb == 0 and k4 == 0),
                            stop=(kb == NKB - 1 and k4 == KC - 1),
                        )
            for mi in range(4):
                m = g * 4 + mi
                nc.scalar.activation(lp[:, m, :], ps[mi][:, 0, :],
                                     Act.Copy, scale=1.0 / S_ENC)
        nc.sync.dma_start(d["ar1i"][:], lp[:])

    # ================= rmsnorm (reload lat, rstd broadcast) =================
    def rms(w):
        d = wv[w]
        lch = latp.tile([P, MD, SL], bf16, name=f"lbfc{w}")
        nc.scalar.dma_start(lch[:], d["ar1o"][:])
        if not ZB_ENC:
            nc.vector.tensor_tensor(
                lch[:], lch[:], cp[:, C_ENCB:C_ENCB + MD][:, :, None]
                .to_broadcast((P, MD, SL)), Alu.add)
        d["lch"] = lch
        lat_bf = [lch[:, m, :] for m in range(MD)]
        d["lat_bf"] = lat_bf
        ss_t = psum.tile([P, SL], f32, name=f"ssps{w}", tag="mm", bufs=2)
        ss_ps = ss_t[0:1, :]
        for m in range(MD):
            sq = scanp.tile([P, SL], bf16, name="sq", tag="sq", bufs=2)
            nc.scalar.activation(sq[:], lat_bf[m], Act.Square)
            nc.tensor.matmul(ss_ps, lhsT=ones_sb[:], rhs=sq[:],
                             start=(m == 0), stop=(m == MD - 1))
        # rstd = exp(-0.5 * ln(ss/D + eps)) (stays in the ln/exp act set)
        lms = scanp.tile([1, SL], f32, name="lms", tag="rmssc", bufs=4)
        nc.scalar.activation(lms[:], ss_ps, Act.Ln, bias=eps_sb[:],
                             scale=1.0 / D_MODEL)
        rstd = scanp.tile([1, SL], f32, name="rstd", tag="rmssc", bufs=4)
        nc.scalar.activation(rstd[:], lms[:], Act.Exp, scale=-0.5)
        rstd_ps = psum.tile([P, SL], f32, name=f"rstdps{w}", tag="mm", bufs=2)
        nc.tensor.matmul(rstd_ps[:], lhsT=ones_row[:], rhs=rstd[:],
                         start=True, stop=True)
        rstd_bc = scanp.tile([P, SL], f32, name=f"rstdbc{w}", tag="rstdbc",
                             bufs=2)
        nc.vector.tensor_copy(rstd_bc[:], rstd_ps[:])
        d["rstd_bc"] = rstd_bc

    # ====== mamba front: full-u in_proj (replicated), z (shard) ======
    def inproj(w):
        d = wv[w]
        uf = ubigp.tile([P, NT, SL], bf16, name=f"ufull{w}", tag="ufull",
                        bufs=1)
        d["uraw"] = uf
        for grp in range(NT // 2):
            ups = psum.tile([P, 2, SL], f32, name=f"ups{w}_{grp}", tag="uzps",
                            bufs=2)
            for i in range(2):
                t = grp * 2 + i
                for k in range(MD):
                    nc.tensor.matmul(
                        ups[:, i, :],
                        lhsT=upk[:, k, t * P:(t + 1) * P],
                        rhs=d["lat_bf"][k], start=(k == 0), stop=(k == MD - 1))
            nc.vector.tensor_tensor(
                uf[:, grp * 2:grp * 2 + 2, :], ups[:],
                d["rstd_bc"][:, None, :].to_broadcast((P, 2, SL)), Alu.mult)
        zps = psum.tile([P, 2, SL], f32, name=f"zps{w}", tag="uzps", bufs=2)
        for i in range(TI):
            for k in range(MD):
                nc.tensor.matmul(
                    zps[:, i, :],
                    lhsT=bp[:, B_IPZ + k * DI_SH + i * P:
                            B_IPZ + k * DI_SH + (i + 1) * P],
                    rhs=d["lat_bf"][k], start=(k == 0), stop=(k == MD - 1))
        zn = scanp.tile([P, TI, SL], f32, name="zn", tag="zn", bufs=1)
        nc.vector.tensor_tensor(
            zn[:], zps[:], d["rstd_bc"][:, None, :].to_broadcast((P, TI, SL)),
            Alu.mult)
        sz = scanp.tile([P, TI, SL], f32, name=f"siluz{w}", tag="sz", bufs=2)
        nc.scalar.activation(sz[:], zn[:], Act.Silu)
        d["silu_z"] = sz

    # ====== causal depthwise conv + silu over the full (permuted) u ======
    def conv_silu(w):
        d = wv[w]
        uf = d["uraw"]
        ub = ubigp.tile([P, NT, SL], bf16, name=f"ubf{w}", tag="ubf", bufs=1)
        d["u_bf"] = ub

        def cw_bc(k, hh, n):
            return bp[:, B_CW + k * NT + hh * 8:B_CW + k * NT + hh * 8 + 8]                 [:, :, None].to_broadcast((P, 8, n))

        for hh in range(NT // 8):
            s8 = slice(hh * 8, hh * 8 + 8)
            acc = ubigp.tile([P, 8, SL], bf16, name="cacc", tag="cacc", bufs=1)
            nc.vector.tensor_tensor(acc[:], uf[:, s8, :], cw_bc(3, hh, SL),
                                    Alu.mult)
            nc.vector.tensor_tensor(
                acc[:], acc[:], bp[:, B_CB + hh * 8:B_CB + hh * 8 + 8]
                [:, :, None].to_broadcast((P, 8, SL)), Alu.add)
            for k in range(3):
                s = 3 - k
                tmp = ubigp.tile([P, 8, SL], bf16, name="ctmp", tag="ctmp",
                                 bufs=1)
                nc.vector.tensor_tensor(tmp[:, :, s:], uf[:, s8, 0:SL - s],
                                        cw_bc(k, hh, SL - s), Alu.mult)
                nc.vector.tensor_tensor(acc[:, :, s:], acc[:, :, s:],
                                        tmp[:, :, s:], Alu.add)
            nc.scalar.activation(ub[:, s8, :], acc[:], Act.Silu)

    # ====== x_proj (local, contraction over full u) + dt + B/C ======
    def xproj(w):
        d = wv[w]
        dbc_t = psum.tile([P, SL], f32, name=f"dbcps{w}", tag="mm", bufs=2)
        dbc_ps = dbc_t[0:96, :]
        for t in range(NT):
            nc.tensor.matmul(dbc_ps, lhsT=bp[:, B_XP + t * 96:
                                             B_XP + (t + 1) * 96],
                             rhs=d["u_bf"][:, t, :], start=(t == 0),
                             stop=(t == NT - 1))
        dbc_bf = scanp.tile([P, SL], bf16, name="dbcbf", tag="dbcbf", bufs=2)
        nc.vector.memset(dbc_bf[:], 0.0)
        nc.vector.tensor_copy(dbc_bf[0:DT_RANK, :], dbc_ps[0:DT_RANK, :])
        bc_bf = scanp.tile([2 * D_STATE, SL], bf16, name="bcbf", tag="bcbf",
                           bufs=2)
        nc.vector.tensor_copy(bc_bf[:], dbc_ps[DT_RANK:DT_RANK + 2 * D_STATE, :])
        nc.sync.dma_start(d["bc_dr"][:], bc_bf[:])

        dt_t = []
        for t in range(TI):
            ps = psum.tile([P, SL], f32, name=f"dtps{w}_{t}", tag="mm", bufs=2)
            nc.tensor.matmul(ps[:], lhsT=bp[:, B_DTP + t * P:
                                            B_DTP + (t + 1) * P],
                             rhs=dbc_bf[:], start=True, stop=True)
            # softplus(x+b) = log1p(exp(x+b)); args well within range
            edt = scanp.tile([P, SL], f32, name="edt", tag="edt", bufs=1)
            nc.scalar.activation(edt[:], ps[:], Act.Exp, bias=c1(C_DTB + t))
            dtt = scanp.tile([P, SL], f32, name=f"dt{w}_{t}", tag="dtt", bufs=4)
            nc.scalar.activation(dtt[:], edt[:], Act.Ln, bias=onesf_sb[:])
            dt_t.append(dtt)
        d["dt_t"] = dt_t

    # ================= selective scan + gate =================
    def scan(w):
        d = wv[w]
        dt_t = d["dt_t"]
        NH = D_STATE // 4           # 4 states per quarter
        NLH = NH * SL               # 1024
        # one broadcast DMA for all of B, one for all of C
        bctB = big.tile([P, D_STATE, SL], bf16, name=f"bctB{w}", tag="bctB",
                        bufs=1)
        nc.scalar.dma_start(bctB[:], d["bc_dr"][0:D_STATE, :][None, :, :]
                            .to_broadcast((P, D_STATE, SL)))
        bctC = big.tile([P, D_STATE, SL], bf16, name=f"bctC{w}", tag="bctC",
                        bufs=1)
        nc.scalar.dma_start(bctC[:], d["bc_dr"][D_STATE:2 * D_STATE, :]
                            [None, :, :].to_broadcast((P, D_STATE, SL)))

        y_t = [scanp.tile([P, SL], f32, name=f"y{w}_{t}", tag="yt", bufs=2)
               for t in range(TI)]
        for t in range(TI):
            du = scanp.tile([P, SL], f32, name="du", tag="du", bufs=2)
            nc.vector.tensor_tensor(du[:], dt_t[t][:], d["u_bf"][:, t, :],
                                    Alu.mult)
            yh = scanp.tile([P, SL], f32, name="yh", tag="du", bufs=2)
            for hf in range(2):
                n0 = hf * 8
                dA = big.tile([P, 8 * SL], bf16, name="dA", tag="dAh", bufs=1)
                dAv = dA[:].rearrange("p (n l) -> p n l", n=8)
                for n in range(8):
                    nc.scalar.activation(dAv[:, n, :], dt_t[t][:], Act.Exp,
                                         scale=cp[:, C_A + t * 16 + n0 + n:
                                                  C_A + t * 16 + n0 + n + 1])
                nc.vector.memset(dAv[:, :, 0:1], 0.0)

                dBu = big.tile([P, 8 * SL], bf16, name="dBu", tag="dBuh",
                               bufs=2)
                nc.gpsimd.tensor_tensor(
                    dBu[:].rearrange("p (n l) -> p n l", n=8),
                    du[:, None, :].to_broadcast((P, 8, SL)),
                    bctB[:, n0:n0 + 8, :], Alu.mult)

                h = big.tile([P, 8 * SL], bf16, name="h", tag="hh", bufs=1)
                nc.vector.tensor_tensor_scan(h[:], dA[:], dBu[:], 0.0,
                                             Alu.mult, Alu.add)

                hC = big.tile([P, 8 * SL], bf16, name="hC", tag="dBuh",
                              bufs=2)
                nc.gpsimd.tensor_tensor(hC[:], h[:], bctC[:, n0:n0 + 8, :]
                                        .rearrange("p n l -> p (n l)"),
                                        Alu.mult)
                tgt = y_t[t][:] if hf == 0 else yh[:]
                nc.vector.tensor_reduce(
                    tgt, hC[:].rearrange("p (n l) -> p l n", n=8),
                    axis=mybir.AxisListType.X, op=Alu.add)
                if hf > 0:
                    nc.vector.tensor_tensor(y_t[t][:], y_t[t][:], yh[:],
                                            Alu.add)
            nc.vector.scalar_tensor_tensor(
                out=y_t[t][:], in0=d["u_bf"][:, t, :], scalar=c1(C_D + t),
                in1=y_t[t][:], op0=Alu.mult, op1=Alu.add)
        d["y_t"] = y_t

        y_bf = []
        for t in range(TI):
            yb16 = scanp.tile([P, SL], bf16, name=f"ybf{w}_{t}", tag="ybf",
                              bufs=2)
            nc.vector.tensor_tensor(yb16[:], y_t[t][:],
                                    d["silu_z"][:, t, :], Alu.mult)
            y_bf.append(yb16)
        d["y_bf"] = y_bf

    def outproj(w):
        d = wv[w]
        hp = latp.tile([P, MD, SL], bf16, name=f"hppar{w}")
        d["hppar"] = hp
        for m in range(MD):
            ps = psum.tile([P, SL], f32, name="mmps", tag="mm", bufs=2)
            for t in range(TI):
                nc.tensor.matmul(
                    ps[:], lhsT=bp[:, B_OP + t * D_MODEL + m * P:
                                   B_OP + t * D_MODEL + (m + 1) * P],
                    rhs=d["y_bf"][t][:], start=(t == 0), stop=(t == TI - 1))
            nc.scalar.activation(hp[:, m, :], ps[:], Act.Copy)
        nc.sync.dma_start(d["ar3i"][:], hp[:])

    # ================= decoder (fp8 DoubleRow) =================
    w_dec = []

    def dec_prefetch():
        for mp in range(NKB):
            dwm = wbig.tile([P, KC, MD, P], fp8, tag="w", bufs=8)
            nc.sync.dma_start(dwm[:],
                              io["dec_wT"][:, KC * mp:KC * mp + KC, :, :])
            w_dec.append(dwm)

    def dec_h(w):
        d = wv[w]
        # reuse dead staging tiles: hppar (after ar3i DMA) for the reload,
        # latpar (after ar1i DMA) for the residual-summed h
        hch = d["hppar"]
        nc.scalar.dma_start(hch[:], d["ar3o"][:])
        lp = d["latpar"]
        nc.vector.tensor_tensor(lp[:], hch[:], d["lch"][:], Alu.add)
        d["h_bf"] = lp

    def dec(w, mps):
        d = wv[w]
        hb = d["h_bf"]
        for mp2 in mps:                     # mp2 indexes pairs of m-chunks
            ot = outp.tile([P, 2 * KC, SL], bf16, name="ot", tag="ot", bufs=2)
            for half in range(2):
                mp = 2 * mp2 + half
                for mi in range(KC):
                    m = KC * mp + mi
                    ps = psum.tile([P, SL], f32, name="mmps", tag="mm", bufs=2)
                    for k in range(MD):
                        nc.tensor.matmul(
                            ps[:], lhsT=w_dec[mp][:, mi, k, :],
                            rhs=hb[:, k, :], start=(k == 0),
                            stop=(k == MD - 1))
                    nc.scalar.activation(ot[:, half * KC + mi, :], ps[:],
                                         Act.Sigmoid, bias=c1(C_DECB + m),
                                         scale=1.0 / S_DEC)
            nc.sync.dma_start(
                io["out"][w, :, 2 * KC * mp2:2 * KC * (mp2 + 1), :], ot[:])

    # ================= emission order (the pipeline) =================
    enc(0)
    ar(wv[0]["ar1i"], wv[0]["ar1o"])
    enc(1)
    ar(wv[1]["ar1i"], wv[1]["ar1o"])
    dec_prefetch()
    rms(0)
    inproj(0)
    conv_silu(0)
    xproj(0)
    if phase == 'enc':
        rms(1)
        for w in range(B):
            d = wv[w]
            for m in range(MD):
                nc.sync.dma_start(io["out"][w, :, m, :], d["lat_bf"][m])
        ctx.close()
        return
    scan(0)
    rms(1)
    inproj(1)
    outproj(0)
    ar(wv[0]["ar3i"], wv[0]["ar3o"])
    conv_silu(1)
    xproj(1)
    scan(1)
    if phase == 'scan':
        for w in range(B):
            d = wv[w]
            for t in range(TI):
                nc.sync.dma_start(io["out"][w, :, t, :], d["y_bf"][t][:])
        ctx.close()
        return
    dec_h(0)
    dec(0, range(2))
    outproj(1)
    ar(wv[1]["ar3i"], wv[1]["ar3o"])
    dec(0, range(2, 4))
    dec_h(1)
    dec(1, range(4))
    ctx.close()


_CACHE = {}
ZB_ENC = True
ZB_CONV = True


def _get_compiled(repeat=1, use_ar=True, phase="all"):
    if ("nc", repeat, use_ar, phase, ZB_ENC, ZB_CONV) in _CACHE:
        return _CACHE[("nc", repeat, use_ar, phase, ZB_ENC, ZB_CONV)]
    nc = bacc.Bacc("TRN2", target_bir_lowering=False, debug=False,
                   num_devices=N_CORES)

    def inp(name, shape, dt=bf16):
        return nc.dram_tensor(name, list(shape), dt, kind="ExternalInput").ap()

    io = {
        "xT": inp("xT", (P, KE, BL), fp8),
        "enc_wT": inp("enc_wT", (P, KE, D_MODEL), fp8),
        "cpack": inp("cpack", (P, C_TOT), f32),
        "bpack": inp("bpack", (P, B_TOT)),
        "upack": inp("upack", (P, MD, D_INNER)),
        "dec_wT": inp("dec_wT", (P, MF, MD, P), fp8),
        "out": nc.dram_tensor("out", [B, P, MF, SL], bf16,
                              kind="ExternalOutput").ap(),
    }
    with tile.TileContext(nc) as tc:
        for _ in range(repeat):
            _body(tc, io, use_ar=use_ar, phase=phase)
    nc.compile()
    _CACHE[("nc", repeat, use_ar, phase, ZB_ENC, ZB_CONV)] = nc
    return nc


def _shard_inputs(x, enc_w, enc_b, dec_w, dec_b, norm_w, in_proj_w, conv_w,
                  conv_b, x_proj_w, dt_proj_w, dt_proj_b, A_log, D_skip,
                  out_proj_w):
    """Host-side preprocessing: transposes, folds, dtype casts, sharding."""
    def q8(a, s):
        a = a * np.float32(s)
        assert np.abs(a).max() < 15.4, np.abs(a).max()
        return a.astype(FP8)

    x2d = np.ascontiguousarray(x.reshape(BL, FLAT).T)          # (FLAT, BL)
    xT = x2d.astype(FP8)
    enc_wT = q8(np.ascontiguousarray(enc_w.T), S_ENC)          # (FLAT, D_MODEL)
    Wp = (in_proj_w * norm_w[None, :])                         # fold rmsnorm scale
    A = -np.exp(A_log).astype(np.float32)                      # (D_INNER, D_STATE)
    dt_projT = np.ascontiguousarray(dt_proj_w.T)               # (64, D_INNER)
    x_projT = np.ascontiguousarray(x_proj_w.T)                 # (D_INNER, 96)
    out_projT = np.ascontiguousarray(out_proj_w.T)             # (D_INNER, D_MODEL)
    conv_w2 = conv_w.reshape(D_CONV, D_INNER)                  # (4, D_INNER)
    Wu = Wp[:D_INNER]                                          # (D_INNER, D_MODEL)

    in_maps = []
    for i in range(N_CORES):
        fsl = slice(i * FLAT_SH, (i + 1) * FLAT_SH)
        dsl = slice(i * DI_SH, (i + 1) * DI_SH)
        # permutation putting this core's scan shard first
        perm = np.concatenate([np.arange(i * DI_SH, (i + 1) * DI_SH),
                               np.arange(0, i * DI_SH),
                               np.arange((i + 1) * DI_SH, D_INNER)])
        Wz = Wp[D_INNER + i * DI_SH: D_INNER + (i + 1) * DI_SH]
        z_projT = np.ascontiguousarray(Wz.T).astype(BF16)      # (D_MODEL, 256)
        dtp = np.zeros((P, DI_SH), np.float32)
        dtp[:DT_RANK] = dt_projT[:, dsl]

        cpack = np.zeros((P, C_TOT), np.float32)
        cpack[:, C_ENCB:C_ENCB + MD] = enc_b.reshape(MD, P).T
        cpack[:, C_DTB:C_DTB + TI] = dt_proj_b[dsl].reshape(TI, P).T
        cpack[:, C_CONVB:C_CONVB + NT] = conv_b[perm].reshape(NT, P).T
        cpack[:, C_A:C_A + 32] = \
            A[dsl].reshape(TI, P, D_STATE).transpose(1, 0, 2).reshape(P, 32)
        cpack[:, C_D:C_D + TI] = D_skip[dsl].reshape(TI, P).T
        cpack[:, C_DECB:C_DECB + MF] = dec_b[fsl].reshape(MF, P).T

        bpack = np.zeros((P, B_TOT), BF16)
        bpack[:, B_XP:B_XP + NT * 96] = \
            x_projT[perm].reshape(NT, P, 96).transpose(1, 0, 2) \
            .reshape(P, NT * 96).astype(BF16)
        bpack[:, B_DTP:B_DTP + DI_SH] = dtp.astype(BF16)
        bpack[:, B_IPZ:B_IPZ + MD * DI_SH] = \
            z_projT.reshape(MD, P, DI_SH).transpose(1, 0, 2) \
            .reshape(P, MD * DI_SH)
        bpack[:, B_OP:B_OP + TI * D_MODEL] = \
            out_projT[dsl].reshape(TI, P, D_MODEL).transpose(1, 0, 2) \
            .reshape(P, TI * D_MODEL).astype(BF16)
        cwp = conv_w2[:, perm].reshape(D_CONV, NT, P)          # (4, NT, P)
        bpack[:, B_CW:B_CW + 4 * NT] = \
            cwp.transpose(2, 0, 1).reshape(P, 4 * NT).astype(BF16)
        bpack[:, B_CB:B_CB + NT] = conv_b[perm].reshape(NT, P).T.astype(BF16)

        upack = np.ascontiguousarray(
            Wu[perm].T.reshape(MD, P, D_INNER).transpose(1, 0, 2)) \
            .astype(BF16)                                      # (P, MD, D_INNER)

        dec_wTi = q8(np.ascontiguousarray(
            dec_w[fsl].reshape(MF, P, MD, P).transpose(3, 0, 2, 1)), S_DEC)
        in_maps.append({
            "xT": np.ascontiguousarray(
                xT[fsl].reshape(KE, P, BL).transpose(1, 0, 2)),
            "enc_wT": np.ascontiguousarray(
                enc_wT[fsl].reshape(KE, P, D_MODEL).transpose(1, 0, 2)),
            "cpack": cpack,
            "bpack": bpack,
            "upack": upack,
            "dec_wT": dec_wTi,
        })
    return in_maps


def run(inputs, trace=False, tmpdir=None):
    """Run on hardware; returns (probs, BassKernelResults)."""
    global ZB_ENC, ZB_CONV
    ZB_ENC = bool(np.all(np.asarray(inputs["enc_b"]) == 0))
    ZB_CONV = bool(np.all(np.asarray(inputs["conv_b"]) == 0))
    in_maps = _shard_inputs(**{k: np.asarray(v) for k, v in inputs.items()})
    nc = _get_compiled()
    res = bass_utils.run_bass_kernel_spmd(
        nc, in_maps, core_ids=list(range(N_CORES)), trace=trace, tmpdir=tmpdir)
    parts = [res.results[c]["out"].transpose(0, 3, 2, 1)   # (B, SL, MF, P)
             .reshape(B, SL, FLAT_SH) for c in range(N_CORES)]
    full = np.concatenate(parts, axis=2).astype(np.float32)    # (B, L, FLAT)
    probs = full.reshape(B, L, H, W)[:, :L - 1]
    return probs, res


def kernel(**inputs):
    return run(inputs, trace=False)[0]
